# revision 20
# baseline (speedup 1.0000x reference)
"""Trainium2 Bass kernel for NRI-style GRU decoder multistep rollout.

Reference math (per batch element b, per step t in 0..T-2):
  ins   = x_t if t <= burn_in_steps else prev_pred
  send  = rel_send @ hidden            [E, GH]
  recv  = rel_rec  @ hidden            [E, GH]
  h1_k  = tanh([send, recv] @ W1_k + b1_k)      k in 0..K-1
  h2_k  = tanh(h1_k @ W2_k + b2_k)
  msgs  = sum_k h2_k * rt[:, k] / K
  agg   = (rel_rec.T @ msgs) / N       [N, MO]
  GRU(ins, agg) -> hidden; pred = ins + MLP(hidden)

Kernel strategy (fully general, no one-hot assumption):
  - data-parallel over B across 8 cores, everything resident in SBUF
  - reassociate fc1: z1 = (hidden @ W_send_k) gathered through rel_send^T etc.
    => z1[km, e] via bf16 PE matmuls streaming rel_send^T / rel_rec^T
  - fc2 with the h1-chunk as the stationary operand => edge-major h2 in
    PSUM (double-buffered); b2 added in place on the vector engine
  - rt-weighting as one vector multiply per group against a host-built
    rtrep replication; aggregation is then ONE matmul per 128-edge chunk
    (stationary = weighted messages, streaming = rel_rec/(K*N) in bf16),
    accumulating the k-halves stacked as [2*MO, Np] — they are folded
    inside the GRU matmuls by row-doubling the GRU weight matrices
  - the group loop is software-pipelined with a one-cycle skew: per cycle
    the PE runs h2(g), z1(g+1), agg(g-1) while scalar runs h2s(g), h1(g+1)
  - GRU r/i gates fused into one [2GH, N] matmul + one sigmoid; all small
    matmuls run as float32r (single-pass, vs 2-pass fp32)
  - out-MLP relu/bias and the pred residual run on the vector engine; the
    next step's hw/z1 prologue is emitted before the out-MLP so the MLP
    overlaps the next step's pipeline start
  - step 0 aggregate (hidden == 0) computed exactly on host
"""

import os
import sys

import numpy as np

for _p in ("/opt/trn_rl_repo", "/root/.axon_site/_ro/trn_rl_repo"):
    if os.path.isdir(_p) and _p not in sys.path:
        sys.path.insert(0, _p)

import ml_dtypes

import concourse.bacc as bacc
import concourse.tile as tile
from concourse import mybir
from concourse.bass_utils import run_bass_kernel_spmd

AF = mybir.ActivationFunctionType
ALU = mybir.AluOpType
F32 = mybir.dt.float32
F32R = mybir.dt.float32r
BF16 = mybir.dt.bfloat16
BF_NP = ml_dtypes.bfloat16


def build_nc(S, Np, GH, K, MH, MO, NH, E_pad, bis, group=1024):
    """Build the per-core Bass program. Returns nc."""
    KM = K * MH   # stacked (k, m) partition dim for h1, must be <= 128
    KO = K * MO   # stacked (k, o) free dim for h2
    assert KM <= 128 and KO <= 128
    assert E_pad % 128 == 0
    n_chunks = E_pad // 128
    group = min(group, E_pad)
    G = (E_pad + group - 1) // group
    gcols = [min(group, E_pad - g * group) for g in range(G)]

    nc = bacc.Bacc("TRN2", target_bir_lowering=False, debug=False)

    def din(name, shape, dt=F32):
        return nc.dram_tensor(name, list(shape), dt, kind="ExternalInput")

    x_d = din("x", [1, S * Np], F32R)
    rels_d = din("rels", [Np, E_pad], BF16)
    relr_d = din("relr", [Np, E_pad], BF16)

    preds_d = nc.dram_tensor("preds", [S, Np], F32, kind="ExternalOutput")
    relrn_d = din("relrn", [128, n_chunks, Np], BF16)
    rtrep_d = din("rtrep", [128, E_pad], BF16)
    w1cat_d = din("w1cat", [GH, 2 * KM], F32R)
    w2bd_d = din("w2bd", [KM, KO], BF16)
    b1_d = din("b1col", [KM, 1])
    b2rep_d = din("b2rep", [128, group])
    ghri_d = din("ghri", [2 * MO, 2 * GH], F32R)
    giri_d = din("giri", [1, 2 * GH], F32R)
    gbri_d = din("gbri", [2 * GH, 1])
    ghn_d = din("ghn", [2 * MO, GH], F32R)
    gin_d = din("gin", [1, GH], F32R)
    gbn_d = din("gbn", [GH, 1])
    of1_d = din("of1", [GH, NH], F32R)
    of2_d = din("of2", [NH, NH], F32R)
    of3_d = din("of3", [NH, 1], F32R)
    ob1_d = din("ob1", [NH, 1])
    ob2_d = din("ob2", [NH, 1])
    ob3_d = din("ob3", [1, 1])
    agg0_d = din("agg0T", [2 * MO, Np], F32R)
    hid0_d = din("hid0", [GH, Np], F32R)

    with tile.TileContext(nc) as tc:
        with (
            tc.tile_pool(name="persist", bufs=1) as pp,
            tc.tile_pool(name="hwp", bufs=2) as hwp,
            tc.tile_pool(name="h1p", bufs=3) as h1p,
            tc.tile_pool(name="h2sp", bufs=2) as h2sp,
            tc.tile_pool(name="wh2sp", bufs=2) as wh2sp,
            tc.tile_pool(name="smp", bufs=2) as smp,
            tc.tile_pool(name="predp", bufs=2) as predp,
            tc.tile_pool(name="z1ps", bufs=1, space="PSUM") as z1ps,
            tc.tile_pool(name="h2ps", bufs=2, space="PSUM") as h2ps,
            tc.tile_pool(name="aggps", bufs=1, space="PSUM") as aggps,
            tc.tile_pool(name="smps", bufs=1, space="PSUM") as smps,
        ):
            # ---- persistent SBUF residents ----
            x_sb = pp.tile([1, S * Np], F32R)
            rel_sT = pp.tile([Np, E_pad], BF16)
            rel_rT = pp.tile([Np, E_pad], BF16)
            relrn = pp.tile([128, n_chunks, Np], BF16)
            rtrep = pp.tile([128, E_pad], BF16)
            w1cat = pp.tile([GH, 2 * KM], F32R)
            w2bd = pp.tile([KM, KO], BF16)
            b1col = pp.tile([KM, 1], F32)
            b2rep = pp.tile([128, group], F32)
            ghri = pp.tile([2 * MO, 2 * GH], F32R)
            giri = pp.tile([1, 2 * GH], F32R)
            gbri = pp.tile([2 * GH, 1], F32)
            ghn = pp.tile([2 * MO, GH], F32R)
            gin = pp.tile([1, GH], F32R)
            gbn = pp.tile([GH, 1], F32)
            of1 = pp.tile([GH, NH], F32R)
            of2 = pp.tile([NH, NH], F32R)
            of3 = pp.tile([NH, 1], F32R)
            ob1 = pp.tile([NH, 1], F32)
            ob2 = pp.tile([NH, 1], F32)
            ob3 = pp.tile([1, 1], F32)
            agg0T = pp.tile([2 * MO, Np], F32R)
            hiddenT = pp.tile([GH, Np], F32R)

            # spread the big loads across queues; the first rel chunks
            # arrive before step-1's first z1 matmuls
            q = E_pad // 4
            for ci in range(4):
                sl = slice(ci * q, (ci + 1) * q)
                nc.sync.dma_start(rel_sT[:, sl], rels_d[:, sl])
                nc.scalar.dma_start(rel_rT[:, sl], relr_d[:, sl])
            half = n_chunks // 2
            nc.sync.dma_start(relrn[:, :half], relrn_d[:, :half])
            nc.scalar.dma_start(relrn[:, half:], relrn_d[:, half:])
            hE = E_pad // 2
            nc.sync.dma_start(rtrep[:, :hE], rtrep_d[:, :hE])
            nc.scalar.dma_start(rtrep[:, hE:], rtrep_d[:, hE:])
            for t_sb, t_d in (
                (x_sb, x_d), (w1cat, w1cat_d), (w2bd, w2bd_d), (b1col, b1_d),
                (b2rep, b2rep_d), (ghri, ghri_d), (giri, giri_d),
                (gbri, gbri_d), (ghn, ghn_d), (gin, gin_d), (gbn, gbn_d),
                (of1, of1_d), (of2, of2_d), (of3, of3_d), (ob1, ob1_d),
                (ob2, ob2_d), (ob3, ob3_d), (agg0T, agg0_d),
                (hiddenT, hid0_d),
            ):
                nc.gpsimd.dma_start(t_sb[:], t_d[:])

            # ---- pipeline stage emitters --------------------------------
            def emit_z1(st):
                """z1(g) matmuls + h1(g) activation for stage st."""
                g = st["g"]
                g0, cols = g * group, gcols[g]
                hw_t = st["hw"]
                z1 = z1ps.tile([128, group], F32, tag="z1")
                for c0 in range(0, cols, 512):
                    w = min(512, cols - c0)
                    nc.tensor.matmul(
                        z1[:KM, c0:c0 + w], hw_t[:, 0:KM],
                        rel_sT[:, g0 + c0:g0 + c0 + w],
                        start=True, stop=False)
                    nc.tensor.matmul(
                        z1[:KM, c0:c0 + w], hw_t[:, KM:2 * KM],
                        rel_rT[:, g0 + c0:g0 + c0 + w],
                        start=False, stop=True)
                h1 = h1p.tile([128, group], BF16, tag="h1")
                nc.scalar.activation(h1[:KM, :cols], z1[:KM, :cols],
                                     AF.Tanh, bias=b1col[:])
                st["h1"] = h1

            def emit_h2(st):
                """h2(g) matmuls; b2 is added in place on the vector engine."""
                g = st["g"]
                cols = gcols[g]
                h1 = st["h1"]
                h2e = h2ps.tile([128, group], F32, tag="h2e")
                for s0 in range(0, cols, 128):
                    nc.tensor.matmul(
                        h2e[:, s0:s0 + KO], h1[:KM, s0:s0 + 128],
                        w2bd[:], start=True, stop=True)
                nc.vector.tensor_add(h2e[:, :cols], h2e[:, :cols],
                                     b2rep[:, :cols])
                st["h2e"] = h2e

            def emit_h2s(st):
                g = st["g"]
                cols = gcols[g]
                h2s = h2sp.tile([128, group], BF16, tag="h2s")
                nc.scalar.activation(h2s[:, :cols], st["h2e"][:, :cols],
                                     AF.Tanh)
                st["h2s"] = h2s

            def emit_wh2s(st):
                """rt-weighted messages: wh2s = h2s * rtrep (one DVE op)."""
                g = st["g"]
                g0, cols = g * group, gcols[g]
                wh2s = wh2sp.tile([128, group], BF16, tag="wh2s")
                nc.vector.tensor_mul(wh2s[:, :cols], st["h2s"][:, :cols],
                                     rtrep[:, g0:g0 + cols])
                st["wh2s"] = wh2s

            def emit_agg(st, agg_ps):
                """one matmul per 128-edge chunk: [128,(k,o)]^T @ relrn."""
                g = st["g"]
                g0, cols = g * group, gcols[g]
                wh2s = st["wh2s"]
                for s0 in range(0, cols, 128):
                    ci = (g0 + s0) // 128
                    nc.tensor.matmul(
                        agg_ps[:], wh2s[:, s0:s0 + 128], relrn[:, ci, :],
                        start=(ci == 0), stop=(ci == n_chunks - 1))

            def emit_prologue():
                """hw matmul + bf16 copy and the first z1/h1 of a step."""
                hw_ps = smps.tile([Np, 2 * KM], F32, tag="sm")
                nc.tensor.matmul(hw_ps[:], hiddenT[:], w1cat[:],
                                 start=True, stop=True)
                hw_t = hwp.tile([Np, 2 * KM], BF16)
                nc.vector.tensor_copy(hw_t[:], hw_ps[:])
                st = {"g": 0, "hw": hw_t}
                emit_z1(st)
                return st

            # ---- the rollout --------------------------------------------
            pred_prev = None
            st_next = None     # pipeline stage carried into step t
            for t in range(S):
                if t <= bis:
                    ins_ap = x_sb[0:1, t * Np:(t + 1) * Np]
                else:
                    ins_ap = pred_prev[:]

                if t == 0:
                    agg_sb = agg0T
                else:
                    stages = {0: st_next}
                    agg_ps = aggps.tile([128, Np], F32)
                    for g in range(G):
                        st = stages[g]
                        emit_h2(st)
                        emit_h2s(st)
                        emit_wh2s(st)
                        if g + 1 < G:
                            st2 = {"g": g + 1, "hw": st["hw"]}
                            emit_z1(st2)
                            stages[g + 1] = st2
                        if g >= 1:
                            emit_agg(stages[g - 1], agg_ps)
                            del stages[g - 1]
                    emit_agg(stages[G - 1], agg_ps)
                    # the k-halves stay unfolded: ghri/ghn are row-doubled so
                    # the GRU matmuls fold them during the contraction
                    agg_sb = smp.tile([2 * MO, Np], F32R, tag="aggsb")
                    nc.vector.tensor_copy(agg_sb[:], agg_ps[:])

                # ---- GRU cell (everything [*, Np] feature-major) ----
                ri_ps = smps.tile([2 * GH, Np], F32, tag="sm")
                nc.tensor.matmul(ri_ps[:], ghri[:], agg_sb[:],
                                 start=True, stop=False)
                nc.tensor.matmul(ri_ps[:], giri[:], ins_ap,
                                 start=False, stop=True)
                ri_sb = smp.tile([2 * GH, Np], F32, tag="ri")
                nc.scalar.activation(ri_sb[:], ri_ps[:], AF.Sigmoid,
                                     bias=gbri[:])
                r_ap = ri_sb[0:GH, :]
                # i gate lives at base partition GH; TensorTensor needs both
                # SBUF inputs at the same base partition, so rebase it to 0
                i0_sb = smp.tile([GH, Np], F32, tag="i0")
                nc.vector.tensor_copy(i0_sb[:], ri_sb[GH:2 * GH, :])

                hn_ps = smps.tile([GH, Np], F32, tag="sm")
                nc.tensor.matmul(hn_ps[:], ghn[:], agg_sb[:], start=True, stop=True)
                rn_sb = smp.tile([GH, Np], F32, tag="rn")
                nc.vector.tensor_mul(rn_sb[:], r_ap, hn_ps[:])
                n2_ps = smps.tile([GH, Np], F32, tag="sm")
                nc.tensor.matmul(n2_ps[:], gin[:], ins_ap, start=True, stop=True)
                npre_sb = smp.tile([GH, Np], F32, tag="npre")
                nc.vector.tensor_add(npre_sb[:], rn_sb[:], n2_ps[:])
                n_sb = smp.tile([GH, Np], F32, tag="n")
                nc.scalar.activation(n_sb[:], npre_sb[:], AF.Tanh, bias=gbn[:])

                d_sb = smp.tile([GH, Np], F32, tag="d")
                nc.vector.tensor_sub(d_sb[:], hiddenT[:], n_sb[:])
                id_sb = smp.tile([GH, Np], F32, tag="id")
                nc.vector.tensor_mul(id_sb[:], i0_sb[:], d_sb[:])
                nc.vector.tensor_add(hiddenT[:], n_sb[:], id_sb[:])

                # next step's hw/z1 prologue overlaps this step's out-MLP
                if t + 1 < S:
                    st_next = emit_prologue()

                # ---- output MLP with residual (relu/bias on vector) ----
                p1_ps = smps.tile([NH, Np], F32, tag="sm")
                nc.tensor.matmul(p1_ps[:], of1[:], hiddenT[:], start=True, stop=True)
                p1_sb = smp.tile([NH, Np], F32R, tag="p1")
                nc.vector.tensor_scalar(p1_sb[:], p1_ps[:], ob1[:], 0.0,
                                        ALU.add, ALU.max)
                p2_ps = smps.tile([NH, Np], F32, tag="sm")
                nc.tensor.matmul(p2_ps[:], of2[:], p1_sb[:], start=True, stop=True)
                p2_sb = smp.tile([NH, Np], F32R, tag="p2")
                nc.vector.tensor_scalar(p2_sb[:], p2_ps[:], ob2[:], 0.0,
                                        ALU.add, ALU.max)
                p3_ps = smps.tile([1, Np], F32, tag="sm")
                nc.tensor.matmul(p3_ps[:], of3[:], p2_sb[:], start=True, stop=True)
                pred_sb = predp.tile([1, Np], F32R, tag="pred")
                nc.vector.scalar_tensor_tensor(pred_sb[:], p3_ps[:], ob3[:],
                                               ins_ap, ALU.add, ALU.add)
                nc.sync.dma_start(preds_d[t:t + 1, :],
                                  pred_sb[:].bitcast(F32))
                pred_prev = pred_sb

    nc.compile()
    return nc


def make_host_inputs(x_b, relsT_bf, relrT_bf, relrn_bf, rel_rec, rt_b,
                     msg_fc1_w, msg_fc1_b, msg_fc2_w, msg_fc2_b,
                     gru_w, out_w, E_pad, group):
    """Build the per-core input map, laid out as SBUF wants."""
    (gru_hr_w, gru_hi_w, gru_hn_w, gru_ir_w, gru_ir_b, gru_ii_w, gru_ii_b,
     gru_in_w, gru_in_b) = gru_w
    (out_fc1_w, out_fc1_b, out_fc2_w, out_fc2_b, out_fc3_w, out_fc3_b) = out_w
    E, N = rel_rec.shape
    K, GH2, MH = msg_fc1_w.shape
    GH = GH2 // 2
    MO = msg_fc2_w.shape[2]
    NH = out_fc1_w.shape[1]
    S, Np = x_b.shape
    n_chunks = E_pad // 128
    f = np.float32
    a = np.ascontiguousarray

    # relrn: rel_rec/(K*N) chunked [128, nch, N] (batch-independent);
    # rtrep: rt replicated across the MO columns of each (chunk, k) block
    rt_pad = np.zeros((E_pad, K), f)
    rt_pad[:E] = rt_b
    rtrep = np.repeat(
        rt_pad.reshape(n_chunks, 128, K).transpose(1, 0, 2), MO, axis=2
    ).reshape(128, E_pad)

    # w1cat columns: [send_k0 | send_k1 | rec_k0 | rec_k1]
    w1cat = np.concatenate(
        [msg_fc1_w[k, :GH, :] for k in range(K)]
        + [msg_fc1_w[k, GH:, :] for k in range(K)], axis=1)
    KM = K * MH
    KO = K * MO
    w2bd = np.zeros((KM, KO), f)
    for k in range(K):
        w2bd[k * MH:(k + 1) * MH, k * MO:(k + 1) * MO] = msg_fc2_w[k]
    b1col = msg_fc1_b.reshape(KM, 1)
    b2row = msg_fc2_b.reshape(1, KO)
    reps = group // 128
    if KO == 128:
        b2rep = np.tile(np.tile(b2row, (128, 1)), (1, reps))
    else:
        pat = np.zeros((128, 128), f)
        pat[:, :KO] = np.tile(b2row, (128, 1))
        b2rep = np.tile(pat, (1, reps))

    # exact step-0 aggregate on host (hidden == 0); the device keeps agg
    # in unfolded [2*MO, Np] form, so pad the bottom half with zeros
    h1c = np.tanh(msg_fc1_b)                     # [K, MH]
    h2c = np.tanh(np.einsum("km,kmo->ko", h1c, msg_fc2_w) + msg_fc2_b)
    wrecsum = np.einsum("en,ek->kn", rel_rec, rt_b) / (K * N)
    agg0T = np.zeros((2 * MO, N), f)
    agg0T[:MO] = np.einsum("ko,kn->on", h2c, wrecsum)

    ghri1 = np.concatenate([gru_hr_w, gru_hi_w], axis=1)     # [MO, 2GH]
    ghri = np.concatenate([ghri1, ghri1], axis=0)            # [2MO, 2GH]
    ghn2 = np.concatenate([gru_hn_w, gru_hn_w], axis=0)      # [2MO, GH]
    giri = np.concatenate([gru_ir_w, gru_ii_w], axis=1).reshape(1, 2 * GH)
    gbri = np.concatenate([gru_ir_b, gru_ii_b]).reshape(2 * GH, 1)

    m = {
        "x": a(x_b.reshape(1, S * Np), f),
        "rels": relsT_bf, "relr": relrT_bf, "relrn": relrn_bf,
        "rtrep": a(rtrep.astype(BF_NP)),
        "w1cat": a(w1cat, f), "w2bd": a(w2bd.astype(BF_NP)),
        "b1col": a(b1col, f), "b2rep": a(b2rep, f),
        "ghri": a(ghri, f), "giri": a(giri, f), "gbri": a(gbri, f),
        "ghn": a(ghn2, f),
        "gin": a(gru_in_w.reshape(1, GH), f),
        "gbn": a(gru_in_b.reshape(GH, 1), f),
        "of1": a(out_fc1_w, f), "of2": a(out_fc2_w, f),
        "of3": a(out_fc3_w.reshape(NH, 1), f),
        "ob1": a(out_fc1_b.reshape(NH, 1), f), "ob2": a(out_fc2_b.reshape(NH, 1), f),
        "ob3": a(out_fc3_b.reshape(1, 1), f),
        "agg0T": a(agg0T, f),
        "hid0": np.zeros((GH, Np), f),
    }
    return m


_NC_CACHE = {}


def _get_nc(S, Np, GH, K, MH, MO, NH, E_pad, bis, group):
    key = (S, Np, GH, K, MH, MO, NH, E_pad, bis, group)
    if key not in _NC_CACHE:
        _NC_CACHE[key] = build_nc(S, Np, GH, K, MH, MO, NH, E_pad, bis, group)
    return _NC_CACHE[key]


def kernel(inputs, rel_rec, rel_send, rel_types,
           msg_fc1_w, msg_fc1_b, msg_fc2_w, msg_fc2_b,
           gru_hr_w, gru_hi_w, gru_hn_w,
           gru_ir_w, gru_ir_b, gru_ii_w, gru_ii_b, gru_in_w, gru_in_b,
           out_fc1_w, out_fc1_b, out_fc2_w, out_fc2_b, out_fc3_w, out_fc3_b,
           burn_in, burn_in_steps, split_len, _trace=False):
    inputs = np.asarray(inputs, np.float32)
    rel_rec = np.asarray(rel_rec, np.float32)
    rel_send = np.asarray(rel_send, np.float32)
    rel_types = np.asarray(rel_types, np.float32)
    B, T, N, F_ = inputs.shape
    E = rel_rec.shape[0]
    K = rel_types.shape[2]
    GH = gru_hr_w.shape[1]
    MH = msg_fc1_w.shape[2]
    MO = msg_fc2_w.shape[2]
    NH = out_fc1_w.shape[1]
    S = T - 1
    bis = int(burn_in_steps)
    E_pad = ((E + 127) // 128) * 128
    group = min(1024, E_pad)

    nc = _get_nc(S, N, GH, K, MH, MO, NH, E_pad, bis, group)

    gru_w = tuple(np.asarray(w, np.float32) for w in (
        gru_hr_w, gru_hi_w, gru_hn_w, gru_ir_w, gru_ir_b, gru_ii_w, gru_ii_b,
        gru_in_w, gru_in_b))
    out_w = tuple(np.asarray(w, np.float32) for w in (
        out_fc1_w, out_fc1_b, out_fc2_w, out_fc2_b, out_fc3_w, out_fc3_b))
    f1w = np.asarray(msg_fc1_w, np.float32)
    f1b = np.asarray(msg_fc1_b, np.float32)
    f2w = np.asarray(msg_fc2_w, np.float32)
    f2b = np.asarray(msg_fc2_b, np.float32)

    # rel matrices are shared across the batch: convert once
    relsT_bf = np.zeros((N, E_pad), BF_NP)
    relsT_bf[:, :E] = rel_send.T.astype(BF_NP)
    relrT_bf = np.zeros((N, E_pad), BF_NP)
    relrT_bf[:, :E] = rel_rec.T.astype(BF_NP)
    E_pad_chunks = E_pad // 128
    relrn_pad = np.zeros((E_pad, N), np.float32)
    relrn_pad[:E] = rel_rec / (K * N)
    relrn_bf = np.ascontiguousarray(
        relrn_pad.reshape(E_pad_chunks, 128, N).transpose(1, 0, 2).astype(BF_NP))

    in_maps = []
    for b in range(B):
        x_b = inputs[b, :S, :, 0]
        in_maps.append(make_host_inputs(
            x_b, relsT_bf, relrT_bf, relrn_bf, rel_rec, rel_types[b],
            f1w, f1b, f2w, f2b, gru_w, out_w, E_pad, group))

    res = run_bass_kernel_spmd(nc, in_maps, core_ids=list(range(B)),
                               trace=_trace)
    out = np.stack([res.results[b]["preds"] for b in range(B)])
    out = out[:, :, :, None].astype(np.float32)
    if _trace:
        return out, res
    return out


# revision 22
# speedup vs baseline: 1.0900x; 1.0900x over previous
"""Trainium2 Bass kernel for NRI-style GRU decoder multistep rollout.

Reference math (per batch element b, per step t in 0..T-2):
  ins   = x_t if t <= burn_in_steps else prev_pred
  send  = rel_send @ hidden            [E, GH]
  recv  = rel_rec  @ hidden            [E, GH]
  h1_k  = tanh([send, recv] @ W1_k + b1_k)      k in 0..K-1
  h2_k  = tanh(h1_k @ W2_k + b2_k)
  msgs  = sum_k h2_k * rt[:, k] / K
  agg   = (rel_rec.T @ msgs) / N       [N, MO]
  GRU(ins, agg) -> hidden; pred = ins + MLP(hidden)

Kernel strategy (fully general, no one-hot assumption):
  - data-parallel over B across 8 cores, everything resident in SBUF
  - reassociate fc1: z1 = (hidden @ W_send_k) gathered through rel_send^T etc.
    => z1[km, e] via bf16 PE matmuls streaming rel_send^T / rel_rec^T
  - fc2 with the h1-chunk as the stationary operand => edge-major h2 in
    PSUM (double-buffered); b2 added in place on the vector engine
  - rt-weighting as one vector multiply per group against a host-built
    rtrep replication; aggregation is then ONE matmul per 128-edge chunk
    (stationary = weighted messages, streaming = rel_rec/(K*N) in bf16),
    accumulating the k-halves stacked as [2*MO, Np] — they are folded
    inside the GRU matmuls by row-doubling the GRU weight matrices
  - the group loop is software-pipelined with a one-cycle skew: per cycle
    the PE runs h2(g), z1(g+1), agg(g-1) while scalar runs h2s(g), h1(g+1)
  - GRU r/i gates fused into one [2GH, N] matmul + one sigmoid; all small
    matmuls run as float32r (single-pass, vs 2-pass fp32)
  - out-MLP relu/bias and the pred residual run on the vector engine; the
    next step's hw/z1 prologue is emitted before the out-MLP so the MLP
    overlaps the next step's pipeline start
  - step 0 aggregate (hidden == 0) computed exactly on host
"""

import os
import sys

import numpy as np

for _p in ("/opt/trn_rl_repo", "/root/.axon_site/_ro/trn_rl_repo"):
    if os.path.isdir(_p) and _p not in sys.path:
        sys.path.insert(0, _p)

import ml_dtypes

import concourse.bacc as bacc
import concourse.tile as tile
from concourse import mybir
from concourse.bass_utils import run_bass_kernel_spmd

AF = mybir.ActivationFunctionType
ALU = mybir.AluOpType
F32 = mybir.dt.float32
F32R = mybir.dt.float32r
BF16 = mybir.dt.bfloat16
BF_NP = ml_dtypes.bfloat16


def build_nc(S, Np, GH, K, MH, MO, NH, E_pad, bis, group=1024):
    """Build the per-core Bass program. Returns nc."""
    KM = K * MH   # stacked (k, m) partition dim for h1, must be <= 128
    KO = K * MO   # stacked (k, o) free dim for h2
    assert KM <= 128 and KO <= 128
    assert E_pad % 128 == 0
    n_chunks = E_pad // 128
    group = min(group, E_pad)
    G = (E_pad + group - 1) // group
    gcols = [min(group, E_pad - g * group) for g in range(G)]

    nc = bacc.Bacc("TRN2", target_bir_lowering=False, debug=False)

    def din(name, shape, dt=F32):
        return nc.dram_tensor(name, list(shape), dt, kind="ExternalInput")

    x_d = din("x", [1, S * Np], F32R)
    rels_d = din("rels", [Np, E_pad], BF16)
    relr_d = din("relr", [Np, E_pad], BF16)

    preds_d = nc.dram_tensor("preds", [S, Np], F32, kind="ExternalOutput")
    relrn_d = din("relrn", [128, n_chunks, Np], BF16)
    rtrep_d = din("rtrep", [128, E_pad], BF16)
    w1cat_d = din("w1cat", [GH, 2 * KM], F32R)
    w2bd_d = din("w2bd", [KM, KO], BF16)
    b1_d = din("b1col", [KM, 1])
    b2rep_d = din("b2rep", [128, group])
    ghri_d = din("ghri", [2 * MO, 2 * GH], F32R)
    giri_d = din("giri", [1, 2 * GH], F32R)
    gbri_d = din("gbri", [2 * GH, 1])
    ghn_d = din("ghn", [2 * MO, GH], F32R)
    gin_d = din("gin", [1, GH], F32R)
    gbn_d = din("gbn", [GH, 1])
    of1_d = din("of1", [GH, NH], F32R)
    of2_d = din("of2", [NH, NH], F32R)
    of3_d = din("of3", [NH, 1], F32R)
    ob1_d = din("ob1", [NH, 1])
    ob2_d = din("ob2", [NH, 1])
    ob3_d = din("ob3", [1, 1])
    agg0_d = din("agg0T", [2 * MO, Np], F32R)
    hid0_d = din("hid0", [GH, Np], F32R)

    with tile.TileContext(nc) as tc:
        with (
            tc.tile_pool(name="persist", bufs=1) as pp,
            tc.tile_pool(name="hwp", bufs=2) as hwp,
            tc.tile_pool(name="h1p", bufs=3) as h1p,
            tc.tile_pool(name="h2sp", bufs=2) as h2sp,
            tc.tile_pool(name="wh2sp", bufs=3) as wh2sp,
            tc.tile_pool(name="smp", bufs=2) as smp,
            tc.tile_pool(name="predp", bufs=2) as predp,
            tc.tile_pool(name="z1ps", bufs=1, space="PSUM") as z1ps,
            tc.tile_pool(name="h2ps", bufs=2, space="PSUM") as h2ps,
            tc.tile_pool(name="aggps", bufs=1, space="PSUM") as aggps,
            tc.tile_pool(name="smps", bufs=1, space="PSUM") as smps,
        ):
            # ---- persistent SBUF residents ----
            x_sb = pp.tile([1, S * Np], F32R)
            rel_sT = pp.tile([Np, E_pad], BF16)
            rel_rT = pp.tile([Np, E_pad], BF16)
            relrn = pp.tile([128, n_chunks, Np], BF16)
            rtrep = pp.tile([128, E_pad], BF16)
            w1cat = pp.tile([GH, 2 * KM], F32R)
            w2bd = pp.tile([KM, KO], BF16)
            b1col = pp.tile([KM, 1], F32)
            b2rep = pp.tile([128, group], F32)
            ghri = pp.tile([2 * MO, 2 * GH], F32R)
            giri = pp.tile([1, 2 * GH], F32R)
            gbri = pp.tile([2 * GH, 1], F32)
            ghn = pp.tile([2 * MO, GH], F32R)
            gin = pp.tile([1, GH], F32R)
            gbn = pp.tile([GH, 1], F32)
            of1 = pp.tile([GH, NH], F32R)
            of2 = pp.tile([NH, NH], F32R)
            of3 = pp.tile([NH, 1], F32R)
            ob1 = pp.tile([NH, 1], F32)
            ob2 = pp.tile([NH, 1], F32)
            ob3 = pp.tile([1, 1], F32)
            agg0T = pp.tile([2 * MO, Np], F32R)
            hiddenT = pp.tile([GH, Np], F32R)

            # spread the big loads across queues; the first rel chunks
            # arrive before step-1's first z1 matmuls
            q = E_pad // 4
            for ci in range(4):
                sl = slice(ci * q, (ci + 1) * q)
                nc.sync.dma_start(rel_sT[:, sl], rels_d[:, sl])
                nc.scalar.dma_start(rel_rT[:, sl], relr_d[:, sl])
            half = n_chunks // 2
            nc.sync.dma_start(relrn[:, :half], relrn_d[:, :half])
            nc.scalar.dma_start(relrn[:, half:], relrn_d[:, half:])
            hE = E_pad // 2
            nc.sync.dma_start(rtrep[:, :hE], rtrep_d[:, :hE])
            nc.scalar.dma_start(rtrep[:, hE:], rtrep_d[:, hE:])
            for t_sb, t_d in (
                (x_sb, x_d), (w1cat, w1cat_d), (w2bd, w2bd_d), (b1col, b1_d),
                (b2rep, b2rep_d), (ghri, ghri_d), (giri, giri_d),
                (gbri, gbri_d), (ghn, ghn_d), (gin, gin_d), (gbn, gbn_d),
                (of1, of1_d), (of2, of2_d), (of3, of3_d), (ob1, ob1_d),
                (ob2, ob2_d), (ob3, ob3_d), (agg0T, agg0_d),
                (hiddenT, hid0_d),
            ):
                nc.gpsimd.dma_start(t_sb[:], t_d[:])

            # ---- pipeline stage emitters --------------------------------
            def emit_z1(st):
                """z1(g) matmuls + h1(g) activation for stage st."""
                g = st["g"]
                g0, cols = g * group, gcols[g]
                hw_t = st["hw"]
                z1 = z1ps.tile([128, group], F32, tag="z1")
                for c0 in range(0, cols, 512):
                    w = min(512, cols - c0)
                    nc.tensor.matmul(
                        z1[:KM, c0:c0 + w], hw_t[:, 0:KM],
                        rel_sT[:, g0 + c0:g0 + c0 + w],
                        start=True, stop=False)
                    nc.tensor.matmul(
                        z1[:KM, c0:c0 + w], hw_t[:, KM:2 * KM],
                        rel_rT[:, g0 + c0:g0 + c0 + w],
                        start=False, stop=True)
                h1 = h1p.tile([128, group], BF16, tag="h1")
                nc.scalar.activation(h1[:KM, :cols], z1[:KM, :cols],
                                     AF.Tanh, bias=b1col[:])
                st["h1"] = h1

            def emit_h2(st, stprev, agg_ps):
                """h2(g) matmuls zipped with agg(g-1); b2 added on vector."""
                g = st["g"]
                cols = gcols[g]
                h1 = st["h1"]
                h2e = h2ps.tile([128, group], F32, tag="h2e")
                pcols = gcols[stprev["g"]] if stprev is not None else 0
                for idx in range(max(cols, pcols) // 128):
                    s0 = idx * 128
                    if s0 < cols:
                        nc.tensor.matmul(
                            h2e[:, s0:s0 + KO], h1[:KM, s0:s0 + 128],
                            w2bd[:], start=True, stop=True)
                    if stprev is not None and s0 < pcols:
                        ci = (stprev["g"] * group + s0) // 128
                        nc.tensor.matmul(
                            agg_ps[:], stprev["wh2s"][:, s0:s0 + 128],
                            relrn[:, ci, :],
                            start=(ci == 0), stop=(ci == n_chunks - 1))
                nc.vector.tensor_add(h2e[:, :cols], h2e[:, :cols],
                                     b2rep[:, :cols])
                st["h2e"] = h2e

            def emit_h2s(st):
                g = st["g"]
                cols = gcols[g]
                h2s = h2sp.tile([128, group], BF16, tag="h2s")
                nc.scalar.activation(h2s[:, :cols], st["h2e"][:, :cols],
                                     AF.Tanh)
                st["h2s"] = h2s

            def emit_wh2s(st):
                """rt-weighted messages: wh2s = h2s * rtrep (one DVE op)."""
                g = st["g"]
                g0, cols = g * group, gcols[g]
                wh2s = wh2sp.tile([128, group], BF16, tag="wh2s")
                nc.vector.tensor_mul(wh2s[:, :cols], st["h2s"][:, :cols],
                                     rtrep[:, g0:g0 + cols])
                st["wh2s"] = wh2s

            def emit_agg(st, agg_ps):
                """one matmul per 128-edge chunk: [128,(k,o)]^T @ relrn."""
                g = st["g"]
                g0, cols = g * group, gcols[g]
                wh2s = st["wh2s"]
                for s0 in range(0, cols, 128):
                    ci = (g0 + s0) // 128
                    nc.tensor.matmul(
                        agg_ps[:], wh2s[:, s0:s0 + 128], relrn[:, ci, :],
                        start=(ci == 0), stop=(ci == n_chunks - 1))

            def emit_prologue():
                """hw matmul + bf16 copy and the first z1/h1 of a step."""
                hw_ps = smps.tile([Np, 2 * KM], F32, tag="sm")
                nc.tensor.matmul(hw_ps[:], hiddenT[:], w1cat[:],
                                 start=True, stop=True)
                hw_t = hwp.tile([Np, 2 * KM], BF16)
                nc.scalar.copy(hw_t[:], hw_ps[:])
                st = {"g": 0, "hw": hw_t}
                emit_z1(st)
                return st

            # ---- the rollout --------------------------------------------
            pred_prev = None
            st_next = None     # pipeline stage carried into step t
            for t in range(S):
                if t <= bis:
                    ins_ap = x_sb[0:1, t * Np:(t + 1) * Np]
                else:
                    ins_ap = pred_prev[:]

                if t == 0:
                    agg_sb = agg0T
                else:
                    stages = {0: st_next}
                    agg_ps = aggps.tile([128, Np], F32)
                    for g in range(G):
                        st = stages[g]
                        if g + 1 < G:
                            st2 = {"g": g + 1, "hw": st["hw"]}
                            emit_z1(st2)
                            stages[g + 1] = st2
                        emit_h2(st, stages.get(g - 1), agg_ps)
                        emit_h2s(st)
                        emit_wh2s(st)
                        if g >= 1:
                            del stages[g - 1]
                    emit_agg(stages[G - 1], agg_ps)
                    # the k-halves stay unfolded: ghri/ghn are row-doubled so
                    # the GRU matmuls fold them during the contraction
                    agg_sb = smp.tile([2 * MO, Np], F32R, tag="aggsb")
                    nc.vector.tensor_copy(agg_sb[:], agg_ps[:])

                # ---- GRU cell (everything [*, Np] feature-major) ----
                ri_ps = smps.tile([2 * GH, Np], F32, tag="sm")
                nc.tensor.matmul(ri_ps[:], ghri[:], agg_sb[:],
                                 start=True, stop=False)
                nc.tensor.matmul(ri_ps[:], giri[:], ins_ap,
                                 start=False, stop=True)
                ri_sb = smp.tile([2 * GH, Np], F32, tag="ri")
                nc.scalar.activation(ri_sb[:], ri_ps[:], AF.Sigmoid,
                                     bias=gbri[:])
                r_ap = ri_sb[0:GH, :]

                hn_ps = smps.tile([GH, Np], F32, tag="sm")
                nc.tensor.matmul(hn_ps[:], ghn[:], agg_sb[:], start=True, stop=True)
                rn_sb = smp.tile([GH, Np], F32, tag="rn")
                nc.vector.tensor_mul(rn_sb[:], r_ap, hn_ps[:])
                n2_ps = smps.tile([GH, Np], F32, tag="sm")
                nc.tensor.matmul(n2_ps[:], gin[:], ins_ap, start=True, stop=True)
                npre_sb = smp.tile([GH, Np], F32, tag="npre")
                nc.vector.tensor_add(npre_sb[:], rn_sb[:], n2_ps[:])
                n_sb = smp.tile([GH, Np], F32, tag="n")
                nc.scalar.activation(n_sb[:], npre_sb[:], AF.Tanh, bias=gbn[:])

                # d lands in the upper partition half so the multiply
                # against the i-gate (at base GH) has equal input bases
                d2_sb = smp.tile([2 * GH, Np], F32, tag="d")
                nc.vector.tensor_sub(d2_sb[GH:2 * GH, :], hiddenT[:], n_sb[:])
                id_sb = smp.tile([GH, Np], F32, tag="id")
                nc.vector.tensor_mul(id_sb[:], ri_sb[GH:2 * GH, :],
                                     d2_sb[GH:2 * GH, :])
                nc.vector.tensor_add(hiddenT[:], n_sb[:], id_sb[:])

                # next step's hw/z1 prologue overlaps this step's out-MLP
                if t + 1 < S:
                    st_next = emit_prologue()

                # ---- output MLP with residual (relu/bias on vector) ----
                p1_ps = smps.tile([NH, Np], F32, tag="sm")
                nc.tensor.matmul(p1_ps[:], of1[:], hiddenT[:], start=True, stop=True)
                p1_sb = smp.tile([NH, Np], F32R, tag="p1")
                nc.vector.tensor_scalar(p1_sb[:], p1_ps[:], ob1[:], 0.0,
                                        ALU.add, ALU.max)
                p2_ps = smps.tile([NH, Np], F32, tag="sm")
                nc.tensor.matmul(p2_ps[:], of2[:], p1_sb[:], start=True, stop=True)
                p2_sb = smp.tile([NH, Np], F32R, tag="p2")
                nc.vector.tensor_scalar(p2_sb[:], p2_ps[:], ob2[:], 0.0,
                                        ALU.add, ALU.max)
                p3_ps = smps.tile([1, Np], F32, tag="sm")
                nc.tensor.matmul(p3_ps[:], of3[:], p2_sb[:], start=True, stop=True)
                pred_sb = predp.tile([1, Np], F32R, tag="pred")
                nc.vector.scalar_tensor_tensor(pred_sb[:], p3_ps[:], ob3[:],
                                               ins_ap, ALU.add, ALU.add)
                nc.sync.dma_start(preds_d[t:t + 1, :],
                                  pred_sb[:].bitcast(F32))
                pred_prev = pred_sb

    nc.compile()
    return nc


def make_host_inputs(x_b, relsT_bf, relrT_bf, relrn_bf, rel_rec, rt_b,
                     msg_fc1_w, msg_fc1_b, msg_fc2_w, msg_fc2_b,
                     gru_w, out_w, E_pad, group):
    """Build the per-core input map, laid out as SBUF wants."""
    (gru_hr_w, gru_hi_w, gru_hn_w, gru_ir_w, gru_ir_b, gru_ii_w, gru_ii_b,
     gru_in_w, gru_in_b) = gru_w
    (out_fc1_w, out_fc1_b, out_fc2_w, out_fc2_b, out_fc3_w, out_fc3_b) = out_w
    E, N = rel_rec.shape
    K, GH2, MH = msg_fc1_w.shape
    GH = GH2 // 2
    MO = msg_fc2_w.shape[2]
    NH = out_fc1_w.shape[1]
    S, Np = x_b.shape
    n_chunks = E_pad // 128
    f = np.float32
    a = np.ascontiguousarray

    # relrn: rel_rec/(K*N) chunked [128, nch, N] (batch-independent);
    # rtrep: rt replicated across the MO columns of each (chunk, k) block
    rt_pad = np.zeros((E_pad, K), f)
    rt_pad[:E] = rt_b
    rtrep = np.repeat(
        rt_pad.reshape(n_chunks, 128, K).transpose(1, 0, 2), MO, axis=2
    ).reshape(128, E_pad)

    # w1cat columns: [send_k0 | send_k1 | rec_k0 | rec_k1]
    w1cat = np.concatenate(
        [msg_fc1_w[k, :GH, :] for k in range(K)]
        + [msg_fc1_w[k, GH:, :] for k in range(K)], axis=1)
    KM = K * MH
    KO = K * MO
    w2bd = np.zeros((KM, KO), f)
    for k in range(K):
        w2bd[k * MH:(k + 1) * MH, k * MO:(k + 1) * MO] = msg_fc2_w[k]
    b1col = msg_fc1_b.reshape(KM, 1)
    b2row = msg_fc2_b.reshape(1, KO)
    reps = group // 128
    if KO == 128:
        b2rep = np.tile(np.tile(b2row, (128, 1)), (1, reps))
    else:
        pat = np.zeros((128, 128), f)
        pat[:, :KO] = np.tile(b2row, (128, 1))
        b2rep = np.tile(pat, (1, reps))

    # exact step-0 aggregate on host (hidden == 0); the device keeps agg
    # in unfolded [2*MO, Np] form, so pad the bottom half with zeros
    h1c = np.tanh(msg_fc1_b)                     # [K, MH]
    h2c = np.tanh(np.einsum("km,kmo->ko", h1c, msg_fc2_w) + msg_fc2_b)
    wrecsum = np.einsum("en,ek->kn", rel_rec, rt_b) / (K * N)
    agg0T = np.zeros((2 * MO, N), f)
    agg0T[:MO] = np.einsum("ko,kn->on", h2c, wrecsum)

    ghri1 = np.concatenate([gru_hr_w, gru_hi_w], axis=1)     # [MO, 2GH]
    ghri = np.concatenate([ghri1, ghri1], axis=0)            # [2MO, 2GH]
    ghn2 = np.concatenate([gru_hn_w, gru_hn_w], axis=0)      # [2MO, GH]
    giri = np.concatenate([gru_ir_w, gru_ii_w], axis=1).reshape(1, 2 * GH)
    gbri = np.concatenate([gru_ir_b, gru_ii_b]).reshape(2 * GH, 1)

    m = {
        "x": a(x_b.reshape(1, S * Np), f),
        "rels": relsT_bf, "relr": relrT_bf, "relrn": relrn_bf,
        "rtrep": a(rtrep.astype(BF_NP)),
        "w1cat": a(w1cat, f), "w2bd": a(w2bd.astype(BF_NP)),
        "b1col": a(b1col, f), "b2rep": a(b2rep, f),
        "ghri": a(ghri, f), "giri": a(giri, f), "gbri": a(gbri, f),
        "ghn": a(ghn2, f),
        "gin": a(gru_in_w.reshape(1, GH), f),
        "gbn": a(gru_in_b.reshape(GH, 1), f),
        "of1": a(out_fc1_w, f), "of2": a(out_fc2_w, f),
        "of3": a(out_fc3_w.reshape(NH, 1), f),
        "ob1": a(out_fc1_b.reshape(NH, 1), f), "ob2": a(out_fc2_b.reshape(NH, 1), f),
        "ob3": a(out_fc3_b.reshape(1, 1), f),
        "agg0T": a(agg0T, f),
        "hid0": np.zeros((GH, Np), f),
    }
    return m


_NC_CACHE = {}


def _get_nc(S, Np, GH, K, MH, MO, NH, E_pad, bis, group):
    key = (S, Np, GH, K, MH, MO, NH, E_pad, bis, group)
    if key not in _NC_CACHE:
        _NC_CACHE[key] = build_nc(S, Np, GH, K, MH, MO, NH, E_pad, bis, group)
    return _NC_CACHE[key]


def kernel(inputs, rel_rec, rel_send, rel_types,
           msg_fc1_w, msg_fc1_b, msg_fc2_w, msg_fc2_b,
           gru_hr_w, gru_hi_w, gru_hn_w,
           gru_ir_w, gru_ir_b, gru_ii_w, gru_ii_b, gru_in_w, gru_in_b,
           out_fc1_w, out_fc1_b, out_fc2_w, out_fc2_b, out_fc3_w, out_fc3_b,
           burn_in, burn_in_steps, split_len, _trace=False):
    inputs = np.asarray(inputs, np.float32)
    rel_rec = np.asarray(rel_rec, np.float32)
    rel_send = np.asarray(rel_send, np.float32)
    rel_types = np.asarray(rel_types, np.float32)
    B, T, N, F_ = inputs.shape
    E = rel_rec.shape[0]
    K = rel_types.shape[2]
    GH = gru_hr_w.shape[1]
    MH = msg_fc1_w.shape[2]
    MO = msg_fc2_w.shape[2]
    NH = out_fc1_w.shape[1]
    S = T - 1
    bis = int(burn_in_steps)
    E_pad = ((E + 127) // 128) * 128
    group = min(1024, E_pad)

    nc = _get_nc(S, N, GH, K, MH, MO, NH, E_pad, bis, group)

    gru_w = tuple(np.asarray(w, np.float32) for w in (
        gru_hr_w, gru_hi_w, gru_hn_w, gru_ir_w, gru_ir_b, gru_ii_w, gru_ii_b,
        gru_in_w, gru_in_b))
    out_w = tuple(np.asarray(w, np.float32) for w in (
        out_fc1_w, out_fc1_b, out_fc2_w, out_fc2_b, out_fc3_w, out_fc3_b))
    f1w = np.asarray(msg_fc1_w, np.float32)
    f1b = np.asarray(msg_fc1_b, np.float32)
    f2w = np.asarray(msg_fc2_w, np.float32)
    f2b = np.asarray(msg_fc2_b, np.float32)

    # rel matrices are shared across the batch: convert once
    relsT_bf = np.zeros((N, E_pad), BF_NP)
    relsT_bf[:, :E] = rel_send.T.astype(BF_NP)
    relrT_bf = np.zeros((N, E_pad), BF_NP)
    relrT_bf[:, :E] = rel_rec.T.astype(BF_NP)
    E_pad_chunks = E_pad // 128
    relrn_pad = np.zeros((E_pad, N), np.float32)
    relrn_pad[:E] = rel_rec / (K * N)
    relrn_bf = np.ascontiguousarray(
        relrn_pad.reshape(E_pad_chunks, 128, N).transpose(1, 0, 2).astype(BF_NP))

    in_maps = []
    for b in range(B):
        x_b = inputs[b, :S, :, 0]
        in_maps.append(make_host_inputs(
            x_b, relsT_bf, relrT_bf, relrn_bf, rel_rec, rel_types[b],
            f1w, f1b, f2w, f2b, gru_w, out_w, E_pad, group))

    res = run_bass_kernel_spmd(nc, in_maps, core_ids=list(range(B)),
                               trace=_trace)
    out = np.stack([res.results[b]["preds"] for b in range(B)])
    out = out[:, :, :, None].astype(np.float32)
    if _trace:
        return out, res
    return out
